# revision 6
# baseline (speedup 1.0000x reference)
"""Trainium2 Bass kernel for nn_DiagonalSSM (LRU-style diagonal complex SSM).

Math: the SSM is linear time-invariant, so y = causal_conv(u, h) with
h[k] = Re(c^H Lam^k b).  Per core (batch-sharded, 32 batches/core) the
4096-step sequence is split into 4 superchunks of L=1024 packed onto the
128 SBUF partitions as (s, b) pairs.  Within a superchunk the causal conv
is computed exactly with block-Toeplitz matmuls (8 distinct 128x128 blocks
of h); cross-superchunk history enters via end-of-superchunk states E
(a matmul against decaying-power matrix P2) transposed on the PE into
X^T = shift32(E)^T and projected through G straight into the conv PSUM
banks.  |Lam|^1024 <= 3.6e-3, so states older than one superchunk are
below the bf16 noise floor and dropped.

All operands are bf16: PSUM accumulation stays f32, the matmul stream is
1 cycle/row either way, and halving the bytes halves the HBM-bound DMA
phase.  The output is stored as bf16 and widened on the host.

Schedule notes (from perfetto traces):
 - Loads are whole tensors only: 2KB-per-partition descriptors sustain
   ~340GB/s across the 16 queues; 1KB descriptors lose ~35%.
 - The HAM clock governor raises the PE 1.2->2.4GHz only after ~5.6us of
   near-dense PE activity, so junk matmuls bridge every DMA wait; gaps
   >~1us reset the governor's accumulator.
 - The two PSUM banks close staggered (A at G_A, B at the final conv
   matmul) so bank A's evacuation and store hide under the tail matmuls,
   and bank B's store is split across both DMA rings.
"""
import numpy as np
import ml_dtypes

import concourse.bass as bass
import concourse.mybir as mybir
import concourse.tile as tile
from concourse import bacc
from concourse.bass_utils import run_bass_kernel_spmd
from concourse.masks import make_identity

B, T, N = 256, 4096, 64
L = 1024            # superchunk length
S = 4               # superchunks packed on partitions
NB = 8              # 128-blocks per superchunk
BLOC = B // 8       # batches per core
NC = 8

F32 = mybir.dt.float32
BF16 = mybir.dt.bfloat16
NPBF16 = ml_dtypes.bfloat16

N_WARM = 10         # 256-col junk matmuls (~213ns each) before E
N_BRIDGE1 = 1       # between E and the transpose (covers t_e evac)
N_BRIDGE2 = 1       # between transpose and convA (covers toep arrival)

_BUILT = {}


def _build_module():
    if "nc" in _BUILT:
        return _BUILT["nc"]
    nc = bacc.Bacc("TRN2", target_bir_lowering=False, debug=False, num_devices=NC)
    ut = nc.dram_tensor("ut", [128, NB * 128], BF16, kind="ExternalInput").ap()
    toep = nc.dram_tensor("toep", [128, NB * 128], BF16,
                          kind="ExternalInput").ap()
    p2sb = nc.dram_tensor("p2sb", [128, NB * 128], BF16,
                          kind="ExternalInput").ap()
    g = nc.dram_tensor("g", [128, L], BF16, kind="ExternalInput").ap()
    y = nc.dram_tensor("y", [128, L], BF16, kind="ExternalOutput").ap()

    with tile.TileContext(nc) as tc:
        with (
            tc.tile_pool(name="sb", bufs=1) as sb,
            tc.tile_pool(name="ps", bufs=1, space="PSUM") as ps,
        ):
            # ---- loads: E pair (ut, p2sb) first, one per ring ----
            t_ut = sb.tile([128, NB * 128], BF16)
            t_toep = sb.tile([128, NB * 128], BF16)
            t_p2 = sb.tile([128, NB * 128], BF16)
            t_g = sb.tile([128, L], BF16)
            nc.sync.dma_start(t_ut[:, :], ut[:, :])
            nc.sync.dma_start(t_toep[:, :], toep[:, :])
            nc.scalar.dma_start(t_p2[:, :], p2sb[:, :])
            nc.scalar.dma_start(t_g[:, :], g[:, :])

            # ---- junk-weights memset first so the PE warm-up starts ASAP;
            # early ACT table touch so the 1.3us table load lands now.
            t_wb = sb.tile([128, 256], BF16)
            nc.vector.memset(t_wb[:, :], 1.0)
            t_ones = sb.tile([1, 32], F32)
            nc.vector.memset(t_ones[:, :], 1.0)
            t_actw = sb.tile([1, 32], F32)
            nc.scalar.copy(t_actw[:, :], t_ones[:, :])
            t_id = sb.tile([128, 128], BF16)
            make_identity(nc, t_id[:, :])
            t_xt = sb.tile([128, 128], BF16)
            t_zf = sb.tile([128, 32], F32)
            nc.vector.memset(t_zf[:, :], 0.0)
            nc.vector.tensor_copy(t_xt[:, 0:32], t_zf[:, :])

            p_junk = ps.tile([128, 256], F32)

            def junk(n):
                for _ in range(n):
                    nc.tensor.matmul(p_junk[:, :], t_wb[:, 0:128],
                                     t_wb[:, 0:256], start=True, stop=True,
                                     skip_group_check=True)

            junk(N_WARM)

            # ---- end-of-superchunk states E = ut^T @ P2 ----
            p_ya = ps.tile([128, 512], F32)
            p_yb = ps.tile([128, 512], F32)
            p_e = ps.tile([128, 128], F32)
            p_xt = ps.tile([128, 96], BF16)
            for jb in range(NB):
                nc.tensor.matmul(
                    p_e[:, :], t_ut[:, 128 * jb:128 * (jb + 1)],
                    t_p2[:, 128 * jb:128 * (jb + 1)],
                    start=(jb == 0), stop=(jb == NB - 1), skip_group_check=True)

            # ---- X = shift32(E): transpose E rows 0:96 into columns 32:128
            # of X^T on the PE; s=0 columns stay zero.
            t_e = sb.tile([128, 128], BF16)
            nc.vector.tensor_copy(t_e[:, :], p_e[:, :])
            junk(N_BRIDGE1)
            nc.tensor.matmul(p_xt[:, :], t_e[0:96, :], t_id[0:96, 0:96],
                             is_transpose=True, start=True, stop=True,
                             skip_group_check=True)
            nc.scalar.copy(t_xt[:, 32:128], p_xt[:, :])
            junk(N_BRIDGE2)

            # ---- block-Toeplitz conv, bank A (cols 0:512) then the
            # toep1-only part of bank B ----
            for jb in range(4):
                nc.tensor.matmul(
                    p_ya[:, 128 * jb:512], t_ut[:, 128 * jb:128 * (jb + 1)],
                    t_toep[:, 0:(4 - jb) * 128],
                    start=(jb == 0), stop=False, skip_group_check=True)
            for jb in range(4, NB):
                nc.tensor.matmul(
                    p_yb[:, 128 * (jb - 4):512],
                    t_ut[:, 128 * jb:128 * (jb + 1)],
                    t_toep[:, 0:(8 - jb) * 128],
                    start=(jb == 4), stop=False, skip_group_check=True)

            # ---- project initial states through G; G_A closes bank A so its
            # evacuation + store overlap the remaining matmuls.
            nc.tensor.matmul(p_ya[:, :], t_xt[:, :], t_g[:, 0:512],
                             start=False, stop=True, skip_group_check=True)
            nc.tensor.matmul(p_yb[:, :], t_xt[:, :], t_g[:, 512:1024],
                             start=False, stop=False, skip_group_check=True)
            # bank-B contributions from the earliest time blocks run last.
            for jb in range(4):
                nc.tensor.matmul(
                    p_yb[:, :], t_ut[:, 128 * jb:128 * (jb + 1)],
                    t_toep[:, (4 - jb) * 128:(8 - jb) * 128],
                    start=False, stop=(jb == 3), skip_group_check=True)

            # ---- evacuate (cast to bf16) + store ----
            # bank A on ACT early; bank B split DVE/ACT, stored on both rings.
            t_y = sb.tile([128, L], BF16)
            nc.scalar.copy(t_y[:, 0:512], p_ya[:, :])
            nc.sync.dma_start(y[:, 0:512], t_y[:, 0:512])
            nc.vector.tensor_copy(t_y[:, 512:768], p_yb[:, 0:256])
            nc.scalar.copy(t_y[:, 768:1024], p_yb[:, 256:512])
            nc.sync.dma_start(y[:, 512:768], t_y[:, 512:768])
            nc.scalar.dma_start(y[:, 768:1024], t_y[:, 768:1024])

    nc.compile()
    _BUILT["nc"] = nc
    return nc


def _make_consts(rho, theta, b_real, b_imag, c_real, c_imag):
    rho = np.asarray(rho, np.float64)
    theta = np.asarray(theta, np.float64)
    r = np.exp(-np.logaddexp(0.0, rho))
    lam = r * np.exp(1j * theta)
    b = np.asarray(b_real, np.float64) + 1j * np.asarray(b_imag, np.float64)
    cconj = np.asarray(c_real, np.float64) - 1j * np.asarray(c_imag, np.float64)

    K = L + 1
    lp = np.empty((K, N), np.complex128)
    lp[0] = 1.0
    for k in range(1, K):
        lp[k] = lp[k - 1] * lam

    h = np.real((cconj * b)[None, :] * lp[:L]).sum(axis=1)

    TOEP = np.zeros((128, NB * 128), np.float64)
    jj = np.arange(128)
    for d in range(NB):
        idx = 128 * d + jj[None, :] - jj[:, None]
        TOEP[:, d * 128:(d + 1) * 128] = np.where(
            idx >= 0, h[np.clip(idx, 0, L - 1)], 0.0)

    P2 = np.empty((L, 128), np.float64)
    bl = b[None, :] * lp[L - 1 - np.arange(L)]
    P2[:, :64] = bl.real
    P2[:, 64:] = bl.imag
    P2SB = P2.reshape(NB, 128, 128).transpose(1, 0, 2).reshape(128, NB * 128)

    gl = cconj[None, :] * lp[1:L + 1]
    G = np.empty((128, L), np.float64)
    G[:64, :] = gl.real.T
    G[64:, :] = -gl.imag.T

    f = lambda x: np.ascontiguousarray(x).astype(NPBF16)
    return f(TOEP), f(P2SB), f(G)


def kernel(u, rho, theta, b_real, b_imag, c_real, c_imag):
    u = np.asarray(u, np.float32)
    TOEP, P2SB, G = _make_consts(rho, theta, b_real, b_imag, c_real, c_imag)
    nc = _build_module()

    in_maps = []
    for c in range(NC):
        uc = u[c * BLOC:(c + 1) * BLOC]                  # (32, 4096)
        utc = np.ascontiguousarray(
            uc.reshape(BLOC, S, NB, 128).transpose(3, 2, 1, 0)
            .reshape(128, NB * 128)).astype(NPBF16)
        in_maps.append({"ut": utc, "toep": TOEP, "p2sb": P2SB, "g": G})

    res = run_bass_kernel_spmd(nc, in_maps, core_ids=list(range(NC)))

    out = np.empty((B, T), np.float32)
    for c in range(NC):
        yc = res.results[c]["y"].astype(np.float32)      # (128, 1024)
        out[c * BLOC:(c + 1) * BLOC] = (
            yc.reshape(S, BLOC, L).transpose(1, 0, 2).reshape(BLOC, T))
    return out


# revision 8
# speedup vs baseline: 1.0644x; 1.0644x over previous
"""Trainium2 Bass kernel for nn_DiagonalSSM (LRU-style diagonal complex SSM).

Math: the SSM is linear time-invariant, so y = causal_conv(u, h) with
h[k] = Re(c^H Lam^k b).  Per core (batch-sharded, 32 batches/core) the
4096-step sequence is split into 4 superchunks of L=1024 packed onto the
128 SBUF partitions as (s, b) pairs.  Within a superchunk the causal conv
is computed exactly with block-Toeplitz matmuls (8 distinct 128x128 blocks
of h); cross-superchunk history enters via end-of-superchunk states E
(a matmul against decaying-power matrix P2) transposed on the PE into
X^T = shift32(E)^T and projected through G straight into the conv PSUM
banks.  |Lam|^1024 <= 3.6e-3, so states older than one superchunk are
below the bf16 noise floor and dropped.

All operands are bf16: PSUM accumulation stays f32, the matmul stream is
1 cycle/row either way, and halving the bytes halves the HBM-bound DMA
phase.  The output is stored as bf16 and widened on the host.

Schedule notes (from perfetto traces):
 - Loads are whole tensors only: 2KB-per-partition descriptors sustain
   ~340GB/s across the 16 queues; 1KB descriptors lose ~35%.
 - The HAM clock governor raises the PE 1.2->2.4GHz only after ~5.6us of
   near-dense PE activity, so junk matmuls bridge every DMA wait; gaps
   >~1us reset the governor's accumulator.
 - The two PSUM banks close staggered (A at G_A, B at the final conv
   matmul) so bank A's evacuation and store hide under the tail matmuls,
   and bank B's store is split across both DMA rings.
"""
import numpy as np
import ml_dtypes

import concourse.bass as bass
import concourse.mybir as mybir
import concourse.tile as tile
from concourse import bacc
from concourse.bass_utils import run_bass_kernel_spmd
from concourse.masks import make_identity

B, T, N = 256, 4096, 64
L = 1024            # superchunk length
S = 4               # superchunks packed on partitions
NB = 8              # 128-blocks per superchunk
BLOC = B // 8       # batches per core
NC = 8

F32 = mybir.dt.float32
BF16 = mybir.dt.bfloat16
NPBF16 = ml_dtypes.bfloat16

# The 16 DMA queues serve both rings' descriptors round-robin, so all four
# loads complete nearly together (~10.3-10.7us); junk must bridge the PE
# from body start (~7.1us) to then, slightly overshooting so the PE never
# idles (an idle->wakeup on a semaphore costs ~450ns extra).
N_WARM = 17         # 256-col junk matmuls (~213ns each) before E
N_BRIDGE1 = 1       # between E and the transpose (covers t_e evac)
N_BRIDGE2 = 0       # toep is resident well before convA by now

_BUILT = {}


def _build_module():
    if "nc" in _BUILT:
        return _BUILT["nc"]
    nc = bacc.Bacc("TRN2", target_bir_lowering=False, debug=False, num_devices=NC)
    ut = nc.dram_tensor("ut", [128, NB * 128], BF16, kind="ExternalInput").ap()
    toep = nc.dram_tensor("toep", [128, NB * 128], BF16,
                          kind="ExternalInput").ap()
    p2sb = nc.dram_tensor("p2sb", [128, NB * 128], BF16,
                          kind="ExternalInput").ap()
    g = nc.dram_tensor("g", [128, L], BF16, kind="ExternalInput").ap()
    y = nc.dram_tensor("y", [128, L], BF16, kind="ExternalOutput").ap()

    with tile.TileContext(nc) as tc:
        with (
            tc.tile_pool(name="sb", bufs=1) as sb,
            tc.tile_pool(name="ps", bufs=1, space="PSUM") as ps,
        ):
            # ---- loads: E pair (ut, p2sb) first, one per ring ----
            t_ut = sb.tile([128, NB * 128], BF16)
            t_toep = sb.tile([128, NB * 128], BF16)
            t_p2 = sb.tile([128, NB * 128], BF16)
            t_g = sb.tile([128, L], BF16)
            nc.sync.dma_start(t_ut[:, :], ut[:, :])
            nc.sync.dma_start(t_toep[:, :], toep[:, :])
            nc.scalar.dma_start(t_p2[:, :], p2sb[:, :])
            nc.scalar.dma_start(t_g[:, :], g[:, :])

            # ---- junk-weights memset first so the PE warm-up starts ASAP;
            # early ACT table touch so the 1.3us table load lands now.
            t_wb = sb.tile([128, 256], BF16)
            nc.vector.memset(t_wb[:, :], 1.0)
            t_ones = sb.tile([1, 32], F32)
            nc.vector.memset(t_ones[:, :], 1.0)
            t_actw = sb.tile([1, 32], F32)
            nc.scalar.copy(t_actw[:, :], t_ones[:, :])
            t_id = sb.tile([128, 128], BF16)
            make_identity(nc, t_id[:, :])
            t_xt = sb.tile([128, 128], BF16)
            t_zf = sb.tile([128, 32], F32)
            nc.vector.memset(t_zf[:, :], 0.0)
            nc.vector.tensor_copy(t_xt[:, 0:32], t_zf[:, :])

            p_junk = ps.tile([128, 256], F32)

            def junk(n):
                for _ in range(n):
                    nc.tensor.matmul(p_junk[:, :], t_wb[:, 0:128],
                                     t_wb[:, 0:256], start=True, stop=True,
                                     skip_group_check=True)

            junk(N_WARM)

            # ---- end-of-superchunk states E = ut^T @ P2 ----
            p_ya = ps.tile([128, 512], F32)
            p_yb = ps.tile([128, 512], F32)
            p_e = ps.tile([128, 128], F32)
            p_xt = ps.tile([128, 96], BF16)
            for jb in range(NB):
                nc.tensor.matmul(
                    p_e[:, :], t_ut[:, 128 * jb:128 * (jb + 1)],
                    t_p2[:, 128 * jb:128 * (jb + 1)],
                    start=(jb == 0), stop=(jb == NB - 1), skip_group_check=True)

            # ---- X = shift32(E): transpose E rows 0:96 into columns 32:128
            # of X^T on the PE; s=0 columns stay zero.
            t_e = sb.tile([128, 128], BF16)
            nc.vector.tensor_copy(t_e[:, :], p_e[:, :])
            junk(N_BRIDGE1)
            nc.tensor.matmul(p_xt[:, :], t_e[0:96, :], t_id[0:96, 0:96],
                             is_transpose=True, start=True, stop=True,
                             skip_group_check=True)
            nc.scalar.copy(t_xt[:, 32:128], p_xt[:, :])
            junk(N_BRIDGE2)

            # ---- block-Toeplitz conv, bank A (cols 0:512) then the
            # toep1-only part of bank B ----
            for jb in range(4):
                nc.tensor.matmul(
                    p_ya[:, 128 * jb:512], t_ut[:, 128 * jb:128 * (jb + 1)],
                    t_toep[:, 0:(4 - jb) * 128],
                    start=(jb == 0), stop=False, skip_group_check=True)
            for jb in range(4, NB):
                nc.tensor.matmul(
                    p_yb[:, 128 * (jb - 4):512],
                    t_ut[:, 128 * jb:128 * (jb + 1)],
                    t_toep[:, 0:(8 - jb) * 128],
                    start=(jb == 4), stop=False, skip_group_check=True)

            # ---- project initial states through G; G_A closes bank A so its
            # evacuation + store overlap the remaining matmuls.
            nc.tensor.matmul(p_ya[:, :], t_xt[:, :], t_g[:, 0:512],
                             start=False, stop=True, skip_group_check=True)
            nc.tensor.matmul(p_yb[:, :], t_xt[:, :], t_g[:, 512:1024],
                             start=False, stop=False, skip_group_check=True)
            # bank-B contributions from the earliest time blocks run last.
            for jb in range(4):
                nc.tensor.matmul(
                    p_yb[:, :], t_ut[:, 128 * jb:128 * (jb + 1)],
                    t_toep[:, (4 - jb) * 128:(8 - jb) * 128],
                    start=False, stop=(jb == 3), skip_group_check=True)

            # ---- evacuate (cast to bf16) + store ----
            # bank A (closed early by G_A) on DVE, stored on sync while the
            # tail matmuls run; bank B split DVE/ACT, stored on scalar.
            t_y = sb.tile([128, L], BF16)
            nc.vector.tensor_copy(t_y[:, 0:512], p_ya[:, :])
            nc.sync.dma_start(y[:, 0:512], t_y[:, 0:512])
            nc.vector.tensor_copy(t_y[:, 512:768], p_yb[:, 0:256])
            nc.scalar.copy(t_y[:, 768:1024], p_yb[:, 256:512])
            nc.scalar.dma_start(y[:, 512:1024], t_y[:, 512:1024])

    nc.compile()
    _BUILT["nc"] = nc
    return nc


def _make_consts(rho, theta, b_real, b_imag, c_real, c_imag):
    rho = np.asarray(rho, np.float64)
    theta = np.asarray(theta, np.float64)
    r = np.exp(-np.logaddexp(0.0, rho))
    lam = r * np.exp(1j * theta)
    b = np.asarray(b_real, np.float64) + 1j * np.asarray(b_imag, np.float64)
    cconj = np.asarray(c_real, np.float64) - 1j * np.asarray(c_imag, np.float64)

    K = L + 1
    lp = np.empty((K, N), np.complex128)
    lp[0] = 1.0
    for k in range(1, K):
        lp[k] = lp[k - 1] * lam

    h = np.real((cconj * b)[None, :] * lp[:L]).sum(axis=1)

    TOEP = np.zeros((128, NB * 128), np.float64)
    jj = np.arange(128)
    for d in range(NB):
        idx = 128 * d + jj[None, :] - jj[:, None]
        TOEP[:, d * 128:(d + 1) * 128] = np.where(
            idx >= 0, h[np.clip(idx, 0, L - 1)], 0.0)

    P2 = np.empty((L, 128), np.float64)
    bl = b[None, :] * lp[L - 1 - np.arange(L)]
    P2[:, :64] = bl.real
    P2[:, 64:] = bl.imag
    P2SB = P2.reshape(NB, 128, 128).transpose(1, 0, 2).reshape(128, NB * 128)

    gl = cconj[None, :] * lp[1:L + 1]
    G = np.empty((128, L), np.float64)
    G[:64, :] = gl.real.T
    G[64:, :] = -gl.imag.T

    f = lambda x: np.ascontiguousarray(x).astype(NPBF16)
    return f(TOEP), f(P2SB), f(G)


def kernel(u, rho, theta, b_real, b_imag, c_real, c_imag):
    u = np.asarray(u, np.float32)
    TOEP, P2SB, G = _make_consts(rho, theta, b_real, b_imag, c_real, c_imag)
    nc = _build_module()

    in_maps = []
    for c in range(NC):
        uc = u[c * BLOC:(c + 1) * BLOC]                  # (32, 4096)
        utc = np.ascontiguousarray(
            uc.reshape(BLOC, S, NB, 128).transpose(3, 2, 1, 0)
            .reshape(128, NB * 128)).astype(NPBF16)
        in_maps.append({"ut": utc, "toep": TOEP, "p2sb": P2SB, "g": G})

    res = run_bass_kernel_spmd(nc, in_maps, core_ids=list(range(NC)))

    out = np.empty((B, T), np.float32)
    for c in range(NC):
        yc = res.results[c]["y"].astype(np.float32)      # (128, 1024)
        out[c * BLOC:(c + 1) * BLOC] = (
            yc.reshape(S, BLOC, L).transpose(1, 0, 2).reshape(BLOC, T))
    return out
